# revision 3
# baseline (speedup 1.0000x reference)
"""Trainium2 Bass kernel for nn_MultiHeadAttention_88210038326473.

Reference computation (B=4, S=2048, HID=2048, H=16, DH=128):
    Q = queries @ Wq.T + bq ; K = keys @ Wk.T + bk ; V = keys @ Wv.T + bv
    per-head scores = Qh Kh^T / sqrt(HID), key-padding + causal mask,
    softmax, out = attn @ Vh, concat heads, + queries residual.

Sharding: 8 cores = 4 batches x 2 head-groups (8 heads each). Each core
computes out[b, :, hg*1024:(hg+1)*1024] (stored transposed [1024, 2048];
host transposes back and assembles).

Device algorithm per core:
  Phase KV: KT = (keys @ Wk.T).T  [1024e, 2048s] and V = keys @ Wv.T
            [2048s, 1024e] -> DRAM scratch (fp32r).
  Phase Q:  QT = (queries @ Wq.T).T [1024e, 2048s] -> resident SBUF.
  Attention per (head, q-chunk of 512): scores computed transposed
            sT[k,q] = KT_h^T QT_h per 128-k-tile (causal tiles only),
            expT = Exp(scale*sT + key_pad_bias) (no max subtraction --
            scores are O(1) so exp never overflows; masked -> exp==0),
            diagonal-band tiles masked by a 0/1 staircase, V-matmul
            accumulates outT[d,q] += V_tile^T expT, row-sums accumulated
            on DVE, partition-summed via ones-matmul, reciprocal,
            broadcast back via K=1 ones-matmul, normalize + residual.

All matmuls use float32r (~13-bit mantissa, full PE rate at N>=512).
"""

import math
import os

import numpy as np

B, S, HID, H, DH = 4, 2048, 2048, 16, 128
NCORES = 8
HPC = 8          # heads per core
EH = HPC * DH    # 1024 e-dims per core
SCALE = 1.0 / math.sqrt(HID)
SC = 256         # projection s-chunk
NSC = S // SC    # 8
QC = 512         # attention q-chunk
NQC = S // QC    # 4
NKT = S // DH    # 16 k-tiles
NEG_BIAS = np.float32(-1.0e30)


def _split_excess_waits(nc, max_waits=1):
    """walrus in this container rejects >1 sem-wait per instruction (CTRL
    lowering). Move excess waits onto preceding NoOps on the same engine."""
    import concourse.mybir as mybir

    n_split = 0
    for fn in nc.m.functions:
        for blk in fn.blocks:
            insts = list(blk.instructions)
            out = []
            changed = False
            for ins in insts:
                si = ins.sync_info
                if si is not None and si.on_wait and len(si.on_wait) > max_waits:
                    waits = list(si.on_wait)
                    carriers, rest = waits[:-max_waits], waits[-max_waits:]
                    for i in range(0, len(carriers), max_waits):
                        chunk = carriers[i : i + max_waits]
                        out.append(
                            mybir.InstNoOp(
                                name=f"{ins.name}-ws{i}",
                                engine=ins.engine,
                                bass_nofuse=True,
                                sync_info=mybir.SyncInfo(on_wait=chunk, on_update=[]),
                            )
                        )
                        n_split += 1
                    ins.sync_info = mybir.SyncInfo(
                        on_wait=rest, on_update=list(si.on_update)
                    )
                    changed = True
                out.append(ins)
            if changed:
                blk.instructions = out
    return n_split


_CACHE = {}


def _build():
    """Build the (core-uniform) Bass program. Returns nc."""
    if "nc" in _CACHE:
        return _CACHE["nc"]

    import concourse.bass as bass
    import concourse.mybir as mybir
    from concourse.tile import TileContext

    F32 = mybir.dt.float32
    F32R = mybir.dt.float32r
    EXP = mybir.ActivationFunctionType.Exp

    nc = bass.Bass("TRN2", target_bir_lowering=False, debug=False)

    qT = nc.dram_tensor("qT", [HID, S], F32R, kind="ExternalInput")
    kT = nc.dram_tensor("kT", [HID, S], F32R, kind="ExternalInput")
    wqT = nc.dram_tensor("wqT", [HID, EH], F32R, kind="ExternalInput")
    wkT = nc.dram_tensor("wkT", [HID, EH], F32R, kind="ExternalInput")
    wvT = nc.dram_tensor("wvT", [HID, EH], F32R, kind="ExternalInput")
    bq_d = nc.dram_tensor("bq_d", [DH, HPC], F32, kind="ExternalInput")
    bk_d = nc.dram_tensor("bk_d", [DH, HPC], F32, kind="ExternalInput")
    bv_d = nc.dram_tensor("bv_d", [1, EH], F32R, kind="ExternalInput")
    kbias_d = nc.dram_tensor("kbias_d", [DH, NKT], F32, kind="ExternalInput")
    stair_d = nc.dram_tensor("stair_d", [DH, QC + 3 * DH], F32R, kind="ExternalInput")
    ones_c_d = nc.dram_tensor("ones_c_d", [DH, 1], F32R, kind="ExternalInput")
    ones_r_d = nc.dram_tensor("ones_r_d", [1, DH], F32R, kind="ExternalInput")
    resid_d = nc.dram_tensor("resid_d", [EH, S], F32, kind="ExternalInput")
    outT_d = nc.dram_tensor("outT_d", [EH, S], F32, kind="ExternalOutput")

    NF = HID // DH  # 16 f-tiles

    with TileContext(nc) as tc, nc.allow_low_precision(reason="fp32r ~ fp32"):
        with tc.tile_pool(name="persist", bufs=1) as persist, \
             tc.tile_pool(name="dram", bufs=1, space="DRAM") as dpool:
            stair = persist.tile([DH, QC + 3 * DH], F32R, tag="stair")
            kbias = persist.tile([DH, NKT], F32, tag="kbias")
            ones_c = persist.tile([DH, 1], F32R, tag="ones_c")
            ones_r = persist.tile([1, DH], F32R, tag="ones_r")
            bq_sb = persist.tile([DH, HPC], F32, tag="bq")
            bk_sb = persist.tile([DH, HPC], F32, tag="bk")
            bv_sb = persist.tile([1, EH], F32R, tag="bv")
            nc.sync.dma_start(stair[:], stair_d[:])
            nc.sync.dma_start(kbias[:], kbias_d[:])
            nc.sync.dma_start(ones_c[:], ones_c_d[:])
            nc.sync.dma_start(ones_r[:], ones_r_d[:])
            nc.sync.dma_start(bq_sb[:], bq_d[:])
            nc.sync.dma_start(bk_sb[:], bk_d[:])
            nc.sync.dma_start(bv_sb[:], bv_d[:])

            ktS = dpool.tile([EH, S], F32R, tag="ktS")
            vS = dpool.tile([S, EH], F32R, tag="vS")

            # ---------------- Phase KV ----------------
            with tc.tile_pool(name="wk", bufs=1) as wkp, \
                 tc.tile_pool(name="wv", bufs=1) as wvp, \
                 tc.tile_pool(name="kc", bufs=2) as kcp, \
                 tc.tile_pool(name="kvstage", bufs=4) as stp, \
                 tc.tile_pool(name="pk", bufs=2, space="PSUM") as pkp, \
                 tc.tile_pool(name="pv", bufs=2, space="PSUM") as pvp:
                wk_t = []
                wv_t = []
                for f in range(NF):
                    t = wkp.tile([DH, EH], F32R, tag=f"wk{f}")
                    nc.sync.dma_start(t[:], wkT[f * DH : (f + 1) * DH, :])
                    wk_t.append(t)
                    t = wvp.tile([DH, EH], F32R, tag=f"wv{f}")
                    nc.sync.dma_start(t[:], wvT[f * DH : (f + 1) * DH, :])
                    wv_t.append(t)

                for sc in range(NSC):
                    s0 = sc * SC
                    kc_t = []
                    for f in range(NF):
                        t = kcp.tile([DH, SC], F32R, tag=f"kc{f}")
                        nc.sync.dma_start(
                            t[:], kT[f * DH : (f + 1) * DH, s0 : s0 + SC]
                        )
                        kc_t.append(t)
                    # K projection: KT[e, s0:s0+SC]
                    for et in range(HPC):
                        pk = pkp.tile([DH, SC], F32)
                        for f in range(NF):
                            nc.tensor.matmul(
                                pk[:],
                                wk_t[f][:, et * DH : (et + 1) * DH],
                                kc_t[f][:],
                                start=(f == 0),
                                stop=(f == NF - 1),
                            )
                        st = stp.tile([DH, SC], F32R, tag="kst")
                        nc.vector.tensor_scalar_add(
                            st[:], pk[:], bk_sb[:, et : et + 1]
                        )
                        nc.sync.dma_start(
                            ktS[et * DH : (et + 1) * DH, s0 : s0 + SC], st[:]
                        )
                    # V projection: V[s0:s0+SC, :]
                    for sti in range(SC // DH):
                        for ec in range(EH // QC):
                            pv = pvp.tile([DH, QC], F32)
                            for f in range(NF):
                                nc.tensor.matmul(
                                    pv[:],
                                    kc_t[f][:, sti * DH : (sti + 1) * DH],
                                    wv_t[f][:, ec * QC : (ec + 1) * QC],
                                    start=(f == 0),
                                    stop=False,
                                )
                            nc.tensor.matmul(
                                pv[:],
                                ones_r[:],
                                bv_sb[:, ec * QC : (ec + 1) * QC],
                                start=False,
                                stop=True,
                            )
                            st = stp.tile([DH, QC], F32R, tag="vst")
                            nc.vector.tensor_copy(st[:], pv[:])
                            nc.sync.dma_start(
                                vS[
                                    s0 + sti * DH : s0 + (sti + 1) * DH,
                                    ec * QC : (ec + 1) * QC,
                                ],
                                st[:],
                            )

            # ---------------- Phase Q (QT stays resident) ----------------
            with tc.tile_pool(name="qt", bufs=1) as qtp:
                qt_t = [
                    qtp.tile([DH, S], F32R, tag=f"qt{et}", name=f"qt{et}")
                    for et in range(HPC)
                ]
                with tc.tile_pool(name="wq", bufs=1) as wqp, \
                     tc.tile_pool(name="qc", bufs=2) as qcp, \
                     tc.tile_pool(name="pq", bufs=4, space="PSUM") as pqp:
                    wq_t = []
                    for f in range(NF):
                        t = wqp.tile([DH, EH], F32R, tag=f"wq{f}")
                        nc.sync.dma_start(t[:], wqT[f * DH : (f + 1) * DH, :])
                        wq_t.append(t)
                    for sc in range(NSC):
                        s0 = sc * SC
                        qc_t = []
                        for f in range(NF):
                            t = qcp.tile([DH, SC], F32R, tag=f"qc{f}")
                            nc.sync.dma_start(
                                t[:], qT[f * DH : (f + 1) * DH, s0 : s0 + SC]
                            )
                            qc_t.append(t)
                        for et in range(HPC):
                            pq = pqp.tile([DH, SC], F32)
                            for f in range(NF):
                                nc.tensor.matmul(
                                    pq[:],
                                    wq_t[f][:, et * DH : (et + 1) * DH],
                                    qc_t[f][:],
                                    start=(f == 0),
                                    stop=(f == NF - 1),
                                )
                            nc.vector.tensor_scalar_add(
                                qt_t[et][:, s0 : s0 + SC],
                                pq[:],
                                bq_sb[:, et : et + 1],
                            )

                # ---------------- Phase attention ----------------
                with tc.tile_pool(name="kvh", bufs=2) as kvhp, \
                     tc.tile_pool(name="ex", bufs=4) as exp_p, \
                     tc.tile_pool(name="acc", bufs=2) as accp, \
                     tc.tile_pool(name="tail", bufs=2) as tailp, \
                     tc.tile_pool(name="outs", bufs=3) as outp_sb, \
                     tc.tile_pool(name="ps_s", bufs=3, space="PSUM") as pss, \
                     tc.tile_pool(name="ps_o", bufs=2, space="PSUM") as pso, \
                     tc.tile_pool(name="ps_t", bufs=1, space="PSUM") as pst, \
                     tc.tile_pool(name="ps_b", bufs=1, space="PSUM") as psb:
                    for h in range(HPC):
                        ktH = kvhp.tile([DH, S], F32R, tag="ktH")
                        nc.sync.dma_start(
                            ktH[:], ktS[h * DH : (h + 1) * DH, :]
                        )
                        vH = kvhp.tile([DH, S], F32R, tag="vH")
                        nc.sync.dma_start(
                            vH[:].rearrange("p (kt d) -> p kt d", kt=NKT),
                            vS[:, h * DH : (h + 1) * DH].rearrange(
                                "(kt p) d -> p kt d", p=DH
                            ),
                        )
                        for qc in range(NQC):
                            q0 = qc * QC
                            nkt = 4 * qc + 4
                            po = pso.tile([DH, QC], F32)
                            acc = accp.tile([DH, QC], F32, tag="acc")
                            for kt in range(nkt):
                                ps = pss.tile([DH, QC], F32)
                                nc.tensor.matmul(
                                    ps[:],
                                    ktH[:, kt * DH : (kt + 1) * DH],
                                    qt_t[h][:, q0 : q0 + QC],
                                    start=True,
                                    stop=True,
                                )
                                ex = exp_p.tile([DH, QC], F32R, tag="ex")
                                nc.scalar.activation(
                                    ex[:],
                                    ps[:],
                                    EXP,
                                    bias=kbias[:, kt : kt + 1],
                                    scale=float(SCALE),
                                )
                                j = kt - 4 * qc
                                if j >= 0:
                                    off = 384 - 128 * j
                                    nc.vector.tensor_mul(
                                        ex[:], ex[:], stair[:, off : off + QC]
                                    )
                                if kt == 0:
                                    nc.vector.tensor_copy(acc[:], ex[:])
                                else:
                                    nc.vector.tensor_add(acc[:], acc[:], ex[:])
                                nc.tensor.matmul(
                                    po[:],
                                    vH[:, kt * DH : (kt + 1) * DH],
                                    ex[:],
                                    start=(kt == 0),
                                    stop=(kt == nkt - 1),
                                )
                            accr = accp.tile([DH, QC], F32R, tag="accr")
                            nc.vector.tensor_copy(accr[:], acc[:])
                            psum = pst.tile([1, QC], F32)
                            nc.tensor.matmul(
                                psum[:], ones_c[:], accr[:], start=True, stop=True
                            )
                            rec = tailp.tile([1, QC], F32R, tag="rec")
                            nc.vector.reciprocal(rec[:], psum[:])
                            pbc = psb.tile([DH, QC], F32)
                            nc.tensor.matmul(
                                pbc[:], ones_r[:], rec[:], start=True, stop=True
                            )
                            bcs = tailp.tile([DH, QC], F32, tag="bcs")
                            nc.vector.tensor_copy(bcs[:], pbc[:])
                            ot = outp_sb.tile([DH, QC], F32, tag="ot")
                            nc.vector.tensor_mul(ot[:], po[:], bcs[:])
                            rq = outp_sb.tile([DH, QC], F32, tag="rq")
                            nc.sync.dma_start(
                                rq[:],
                                resid_d[h * DH : (h + 1) * DH, q0 : q0 + QC],
                            )
                            nc.vector.tensor_add(ot[:], ot[:], rq[:])
                            nc.sync.dma_start(
                                outT_d[h * DH : (h + 1) * DH, q0 : q0 + QC], ot[:]
                            )

    _split_excess_waits(nc, max_waits=1)
    _CACHE["nc"] = nc
    return nc


def _host_prep(queries, keys, Wq, bq, Wk, bk, Wv, bv):
    """Build the 8 per-core input maps (host-side shard + layout prep)."""
    queries = np.ascontiguousarray(queries, dtype=np.float32)
    keys = np.ascontiguousarray(keys, dtype=np.float32)

    qT = np.ascontiguousarray(queries.transpose(0, 2, 1))  # [B, HID, S]
    kT = np.ascontiguousarray(keys.transpose(0, 2, 1))
    WqT = np.ascontiguousarray(np.asarray(Wq, np.float32).T)  # [f, e]
    WkT = np.ascontiguousarray(np.asarray(Wk, np.float32).T)
    WvT = np.ascontiguousarray(np.asarray(Wv, np.float32).T)
    bq = np.asarray(bq, np.float32)
    bk = np.asarray(bk, np.float32)
    bv = np.asarray(bv, np.float32)

    # key padding mask -> additive bias per (b, k): 0 keep, -1e30 mask
    ksum = keys.sum(axis=-1)  # [B, S]
    kbias_all = np.where(ksum != 0.0, np.float32(0), NEG_BIAS).astype(np.float32)

    # staircase causal mask for the 4 diagonal-band k-tiles of a 512-q-chunk:
    # mask_j[p, q'] = G[p, q' + 384 - 128*j]
    W = QC + 3 * DH
    x = np.arange(W)[None, :]
    p = np.arange(DH)[:, None]
    stair = np.where(
        x < 384, 0.0, np.where(x >= 512, 1.0, (x - 384) >= p)
    ).astype(np.float32)

    ones_c = np.ones((DH, 1), np.float32)
    ones_r = np.ones((1, DH), np.float32)

    in_maps = []
    for c in range(NCORES):
        b, hg = divmod(c, 2)
        e0 = hg * EH
        in_maps.append(
            {
                "qT": qT[b],
                "kT": kT[b],
                "wqT": np.ascontiguousarray(WqT[:, e0 : e0 + EH]),
                "wkT": np.ascontiguousarray(WkT[:, e0 : e0 + EH]),
                "wvT": np.ascontiguousarray(WvT[:, e0 : e0 + EH]),
                "bq_d": np.ascontiguousarray(
                    bq[e0 : e0 + EH].reshape(HPC, DH).T
                ),
                "bk_d": np.ascontiguousarray(
                    bk[e0 : e0 + EH].reshape(HPC, DH).T
                ),
                "bv_d": np.ascontiguousarray(bv[e0 : e0 + EH].reshape(1, EH)),
                "kbias_d": np.ascontiguousarray(
                    kbias_all[b].reshape(NKT, DH).T
                ),
                "stair_d": stair,
                "ones_c_d": ones_c,
                "ones_r_d": ones_r,
                "resid_d": np.ascontiguousarray(qT[b][e0 : e0 + EH, :]),
            }
        )
    return in_maps


def _assemble(results):
    """results: list of 8 dicts with outT_d [EH, S] -> full [B, S, HID]."""
    out = np.empty((B, S, HID), np.float32)
    for c in range(NCORES):
        b, hg = divmod(c, 2)
        out[b, :, hg * EH : (hg + 1) * EH] = results[c]["outT_d"].T
    return out


def kernel(**inputs):
    from concourse.bass_utils import run_bass_kernel_spmd

    nc = _build()
    in_maps = _host_prep(**inputs)
    res = run_bass_kernel_spmd(nc, in_maps, core_ids=list(range(NCORES)))
    kernel.last_results = res
    return _assemble(res.results)


# revision 4
# speedup vs baseline: 3.6744x; 3.6744x over previous
"""Trainium2 Bass kernel for nn_MultiHeadAttention_88210038326473.

Reference computation (B=4, S=2048, HID=2048, H=16, DH=128):
    Q = queries @ Wq.T + bq ; K = keys @ Wk.T + bk ; V = keys @ Wv.T + bv
    per-head scores = Qh Kh^T / sqrt(HID), key-padding + causal mask,
    softmax, out = attn @ Vh, concat heads, + queries residual.

Sharding: 8 cores = 4 batches x 2 head-groups (8 heads each). Each core
computes out[b, :, hg*1024:(hg+1)*1024] (stored transposed [1024, 2048];
host transposes back and assembles).

Device algorithm per core:
  Phase KV: KT = (keys @ Wk.T).T  [1024e, 2048s] and V = keys @ Wv.T
            [2048s, 1024e] -> DRAM scratch (fp32r).
  Phase Q:  QT = (queries @ Wq.T).T [1024e, 2048s] -> resident SBUF.
  Attention per (head, q-chunk of 512): scores computed transposed
            sT[k,q] = KT_h^T QT_h per 128-k-tile (causal tiles only; the
            diagonal-band tiles only compute the valid right part),
            expT = Exp(scale*sT + key_pad_bias) (no max subtraction --
            scores are O(1) so exp never overflows; masked -> exp==0),
            diagonal 128x128 blocks masked by a 0/1 triangle, V-matmul
            accumulates outT[d,q] += V_tile^T expT, row-sums accumulated
            in PSUM via a ones-column matmul, reciprocal, broadcast back
            via a K=1 ones-matmul, normalize + residual.

All matmuls use float32r (~13-bit mantissa, full PE rate at N>=256).
"""

import math

import numpy as np

B, S, HID, H, DH = 4, 2048, 2048, 16, 128
NCORES = 8
HPC = 8          # heads per core
EH = HPC * DH    # 1024 e-dims per core
SCALE = 1.0 / math.sqrt(HID)
SC = 256         # projection s-chunk
NSC = S // SC    # 8
QC = 512         # attention q-chunk
NQC = S // QC    # 4
NKT = S // DH    # 16 k-tiles
NF = HID // DH   # 16 f-tiles (contraction)
NEG_BIAS = np.float32(-1.0e30)


def _split_excess_waits(nc, max_waits=1):
    """walrus in this container rejects >1 sem-wait per instruction (CTRL
    lowering). Move excess waits onto preceding NoOps on the same engine."""
    import concourse.mybir as mybir

    n_split = 0
    for fn in nc.m.functions:
        for blk in fn.blocks:
            insts = list(blk.instructions)
            out = []
            changed = False
            for ins in insts:
                si = ins.sync_info
                if si is not None and si.on_wait and len(si.on_wait) > max_waits:
                    waits = list(si.on_wait)
                    carriers, rest = waits[:-max_waits], waits[-max_waits:]
                    for i in range(0, len(carriers), max_waits):
                        chunk = carriers[i : i + max_waits]
                        out.append(
                            mybir.InstNoOp(
                                name=f"{ins.name}-ws{i}",
                                engine=ins.engine,
                                bass_nofuse=True,
                                sync_info=mybir.SyncInfo(on_wait=chunk, on_update=[]),
                            )
                        )
                        n_split += 1
                    ins.sync_info = mybir.SyncInfo(
                        on_wait=rest, on_update=list(si.on_update)
                    )
                    changed = True
                out.append(ins)
            if changed:
                blk.instructions = out
    return n_split


_CACHE = {}


def _build():
    """Build the (core-uniform) Bass program. Returns nc."""
    if "nc" in _CACHE:
        return _CACHE["nc"]

    import concourse.bass as bass
    import concourse.mybir as mybir
    from concourse.tile import TileContext

    F32 = mybir.dt.float32
    F32R = mybir.dt.float32r
    EXP = mybir.ActivationFunctionType.Exp
    IDENT = mybir.ActivationFunctionType.Identity

    nc = bass.Bass("TRN2", target_bir_lowering=False, debug=False)

    qT = nc.dram_tensor("qT", [HID, S], F32R, kind="ExternalInput")
    kT = nc.dram_tensor("kT", [HID, S], F32R, kind="ExternalInput")
    wqT = nc.dram_tensor("wqT", [HID, EH], F32R, kind="ExternalInput")
    wkT = nc.dram_tensor("wkT", [HID, EH], F32R, kind="ExternalInput")
    wvT = nc.dram_tensor("wvT", [HID, EH], F32R, kind="ExternalInput")
    bq_d = nc.dram_tensor("bq_d", [DH, HPC], F32, kind="ExternalInput")
    bk_d = nc.dram_tensor("bk_d", [DH, HPC], F32, kind="ExternalInput")
    bv_d = nc.dram_tensor("bv_d", [1, EH], F32R, kind="ExternalInput")
    kbias_d = nc.dram_tensor("kbias_d", [DH, NKT], F32, kind="ExternalInput")
    tri_d = nc.dram_tensor("tri_d", [DH, DH], F32R, kind="ExternalInput")
    ones_c_d = nc.dram_tensor("ones_c_d", [DH, 1], F32R, kind="ExternalInput")
    ones_r_d = nc.dram_tensor("ones_r_d", [1, DH], F32R, kind="ExternalInput")
    resid_d = nc.dram_tensor("resid_d", [EH, S], F32, kind="ExternalInput")
    outT_d = nc.dram_tensor("outT_d", [EH, S], F32, kind="ExternalOutput")

    # 3D views with the 128-partition dim innermost on rows
    qT3 = qT[:].rearrange("(f p) s -> p f s", p=DH)
    kT3 = kT[:].rearrange("(f p) s -> p f s", p=DH)
    wq3 = wqT[:].rearrange("(f p) e -> p f e", p=DH)
    wk3 = wkT[:].rearrange("(f p) e -> p f e", p=DH)
    wv3 = wvT[:].rearrange("(f p) e -> p f e", p=DH)

    with TileContext(nc) as tc, nc.allow_low_precision(reason="fp32r ~ fp32"):
        with tc.tile_pool(name="persist", bufs=1) as persist, \
             tc.tile_pool(name="dram", bufs=1, space="DRAM") as dpool:
            tri = persist.tile([DH, DH], F32R, tag="tri")
            kbias = persist.tile([DH, NKT], F32, tag="kbias")
            ones_c = persist.tile([DH, 1], F32R, tag="ones_c")
            ones_r = persist.tile([1, DH], F32R, tag="ones_r")
            bq_sb = persist.tile([DH, HPC], F32, tag="bq")
            bk_sb = persist.tile([DH, HPC], F32, tag="bk")
            bv_sb = persist.tile([1, EH], F32R, tag="bv")
            nc.sync.dma_start(tri[:], tri_d[:])
            nc.sync.dma_start(kbias[:], kbias_d[:])
            nc.sync.dma_start(ones_c[:], ones_c_d[:])
            nc.sync.dma_start(ones_r[:], ones_r_d[:])
            nc.sync.dma_start(bq_sb[:], bq_d[:])
            nc.sync.dma_start(bk_sb[:], bk_d[:])
            nc.sync.dma_start(bv_sb[:], bv_d[:])

            ktS = dpool.tile([EH, S], F32R, tag="ktS")
            vS = dpool.tile([S, EH], F32R, tag="vS")
            ktS3 = ktS[:].rearrange("(et p) s -> p et s", p=DH)

            # ---------------- Phase KV ----------------
            with tc.tile_pool(name="wk", bufs=1) as wkp, \
                 tc.tile_pool(name="wv", bufs=1) as wvp, \
                 tc.tile_pool(name="kc", bufs=2) as kcp, \
                 tc.tile_pool(name="kvstage", bufs=3) as stp, \
                 tc.tile_pool(name="pk", bufs=2, space="PSUM") as pkp, \
                 tc.tile_pool(name="pv", bufs=2, space="PSUM") as pvp:
                wk_t = wkp.tile([DH, NF * EH], F32R, tag="wk", name="wk")
                nc.sync.dma_start(
                    wk_t[:].rearrange("p (f e) -> p f e", f=NF), wk3
                )
                wv_t = wvp.tile([DH, NF * EH], F32R, tag="wv", name="wv")
                nc.sync.dma_start(
                    wv_t[:].rearrange("p (f e) -> p f e", f=NF), wv3
                )

                for sc in range(NSC):
                    s0 = sc * SC
                    kc = kcp.tile([DH, NF * SC], F32R, tag="kc", name="kc")
                    nc.sync.dma_start(
                        kc[:].rearrange("p (f s) -> p f s", f=NF),
                        kT3[:, :, s0 : s0 + SC],
                    )
                    # K projection: KT[:, s0:s0+SC] for all 8 e-tiles
                    kst = stp.tile([DH, HPC * SC], F32R, tag="kst", name="kst")
                    for et in range(HPC):
                        pk = pkp.tile([DH, SC], F32, name="pk")
                        for f in range(NF):
                            nc.tensor.matmul(
                                pk[:],
                                wk_t[:, f * EH + et * DH : f * EH + (et + 1) * DH],
                                kc[:, f * SC : (f + 1) * SC],
                                start=(f == 0),
                                stop=(f == NF - 1),
                            )
                        nc.scalar.activation(
                            kst[:, et * SC : (et + 1) * SC],
                            pk[:],
                            IDENT,
                            bias=bk_sb[:, et : et + 1],
                        )
                    nc.sync.dma_start(
                        ktS3[:, :, s0 : s0 + SC],
                        kst[:].rearrange("p (et s) -> p et s", et=HPC),
                    )
                    # V projection: V[s0:s0+SC, :]
                    for sti in range(SC // DH):
                        vst = stp.tile([DH, EH], F32R, tag="vst", name="vst")
                        for ec in range(EH // QC):
                            pv = pvp.tile([DH, QC], F32, name="pv")
                            for f in range(NF):
                                nc.tensor.matmul(
                                    pv[:],
                                    kc[:, f * SC + sti * DH : f * SC + (sti + 1) * DH],
                                    wv_t[:, f * EH + ec * QC : f * EH + (ec + 1) * QC],
                                    start=(f == 0),
                                    stop=False,
                                )
                            nc.tensor.matmul(
                                pv[:],
                                ones_r[:],
                                bv_sb[:, ec * QC : (ec + 1) * QC],
                                start=False,
                                stop=True,
                            )
                            nc.scalar.copy(vst[:, ec * QC : (ec + 1) * QC], pv[:])
                        nc.sync.dma_start(
                            vS[s0 + sti * DH : s0 + (sti + 1) * DH, :], vst[:]
                        )

            # ---------------- Phase Q (QT stays resident) ----------------
            with tc.tile_pool(name="qt", bufs=1) as qtp:
                qt_t = [
                    qtp.tile([DH, S], F32R, tag=f"qt{et}", name=f"qt{et}")
                    for et in range(HPC)
                ]
                with tc.tile_pool(name="wq", bufs=1) as wqp, \
                     tc.tile_pool(name="qc", bufs=2) as qcp, \
                     tc.tile_pool(name="pq", bufs=4, space="PSUM") as pqp:
                    wq_t = wqp.tile([DH, NF * EH], F32R, tag="wq", name="wq")
                    nc.sync.dma_start(
                        wq_t[:].rearrange("p (f e) -> p f e", f=NF), wq3
                    )
                    for sc in range(NSC):
                        s0 = sc * SC
                        qch = qcp.tile([DH, NF * SC], F32R, tag="qch", name="qch")
                        nc.sync.dma_start(
                            qch[:].rearrange("p (f s) -> p f s", f=NF),
                            qT3[:, :, s0 : s0 + SC],
                        )
                        for et in range(HPC):
                            pq = pqp.tile([DH, SC], F32, name="pq")
                            for f in range(NF):
                                nc.tensor.matmul(
                                    pq[:],
                                    wq_t[:, f * EH + et * DH : f * EH + (et + 1) * DH],
                                    qch[:, f * SC : (f + 1) * SC],
                                    start=(f == 0),
                                    stop=(f == NF - 1),
                                )
                            nc.scalar.activation(
                                qt_t[et][:, s0 : s0 + SC],
                                pq[:],
                                IDENT,
                                bias=bq_sb[:, et : et + 1],
                            )

                # ---------------- Phase attention ----------------
                with tc.tile_pool(name="kvh", bufs=2) as kvhp, \
                     tc.tile_pool(name="ex", bufs=4) as exp_p, \
                     tc.tile_pool(name="tail", bufs=2) as tailp, \
                     tc.tile_pool(name="outs", bufs=2) as outp_sb, \
                     tc.tile_pool(name="ps_s", bufs=3, space="PSUM") as pss, \
                     tc.tile_pool(name="ps_o", bufs=2, space="PSUM") as pso, \
                     tc.tile_pool(name="ps_t", bufs=2, space="PSUM") as pst, \
                     tc.tile_pool(name="ps_b", bufs=1, space="PSUM") as psb:
                    for h in range(HPC):
                        ktH = kvhp.tile([DH, S], F32R, tag="ktH", name="ktH")
                        nc.sync.dma_start(ktH[:], ktS[h * DH : (h + 1) * DH, :])
                        vH = kvhp.tile([DH, S], F32R, tag="vH", name="vH")
                        nc.sync.dma_start(
                            vH[:].rearrange("p (kt d) -> p kt d", kt=NKT),
                            vS[:, h * DH : (h + 1) * DH].rearrange(
                                "(kt p) d -> p kt d", p=DH
                            ),
                        )
                        rsd = outp_sb.tile([DH, S], F32, tag="rsd", name="rsd")
                        nc.sync.dma_start(
                            rsd[:], resid_d[h * DH : (h + 1) * DH, :]
                        )
                        oth = outp_sb.tile([DH, S], F32, tag="oth", name="oth")
                        for qc in range(NQC):
                            q0 = qc * QC
                            nkt = 4 * qc + 4
                            po = pso.tile([DH, QC], F32, name="po")
                            psum = pst.tile([1, QC], F32, name="psum")
                            for kt in range(nkt):
                                j = kt - 4 * qc
                                off = max(j, 0) * DH  # valid q-cols start
                                w = QC - off
                                ps = pss.tile([DH, QC], F32, name="ps")
                                nc.tensor.matmul(
                                    ps[:, off:QC],
                                    ktH[:, kt * DH : (kt + 1) * DH],
                                    qt_t[h][:, q0 + off : q0 + QC],
                                    start=True,
                                    stop=True,
                                )
                                ex = exp_p.tile([DH, QC], F32R, tag="ex", name="ex")
                                nc.scalar.activation(
                                    ex[:, off:QC],
                                    ps[:, off:QC],
                                    EXP,
                                    bias=kbias[:, kt : kt + 1],
                                    scale=float(SCALE),
                                )
                                if j >= 0:
                                    # diagonal 128x128 block: causal triangle
                                    nc.vector.tensor_mul(
                                        ex[:, off : off + DH],
                                        ex[:, off : off + DH],
                                        tri[:],
                                    )
                                nc.tensor.matmul(
                                    po[:, off:QC],
                                    vH[:, kt * DH : (kt + 1) * DH],
                                    ex[:, off:QC],
                                    start=(kt == 0),
                                    stop=(kt == nkt - 1),
                                )
                                nc.tensor.matmul(
                                    psum[:, off:QC],
                                    ones_c[:],
                                    ex[:, off:QC],
                                    start=(kt == 0),
                                    stop=(kt == nkt - 1),
                                )
                            rec = tailp.tile([1, QC], F32R, tag="rec", name="rec")
                            nc.vector.reciprocal(rec[:], psum[:])
                            pbc = psb.tile([DH, QC], F32, name="pbc")
                            nc.tensor.matmul(
                                pbc[:], ones_r[:], rec[:], start=True, stop=True
                            )
                            bcs = tailp.tile([DH, QC], F32, tag="bcs", name="bcs")
                            nc.vector.tensor_copy(bcs[:], pbc[:])
                            nc.vector.tensor_mul(
                                oth[:, q0 : q0 + QC], po[:], bcs[:]
                            )
                            nc.vector.tensor_add(
                                oth[:, q0 : q0 + QC],
                                oth[:, q0 : q0 + QC],
                                rsd[:, q0 : q0 + QC],
                            )
                        nc.sync.dma_start(
                            outT_d[h * DH : (h + 1) * DH, :], oth[:]
                        )

    _split_excess_waits(nc, max_waits=1)
    _CACHE["nc"] = nc
    return nc


def _host_prep(queries, keys, Wq, bq, Wk, bk, Wv, bv):
    """Build the 8 per-core input maps (host-side shard + layout prep)."""
    queries = np.ascontiguousarray(queries, dtype=np.float32)
    keys = np.ascontiguousarray(keys, dtype=np.float32)

    qT = np.ascontiguousarray(queries.transpose(0, 2, 1))  # [B, HID, S]
    kT = np.ascontiguousarray(keys.transpose(0, 2, 1))
    WqT = np.ascontiguousarray(np.asarray(Wq, np.float32).T)  # [f, e]
    WkT = np.ascontiguousarray(np.asarray(Wk, np.float32).T)
    WvT = np.ascontiguousarray(np.asarray(Wv, np.float32).T)
    bq = np.asarray(bq, np.float32)
    bk = np.asarray(bk, np.float32)
    bv = np.asarray(bv, np.float32)

    # key padding mask -> additive bias per (b, k): 0 keep, -1e30 mask
    ksum = keys.sum(axis=-1)  # [B, S]
    kbias_all = np.where(ksum != 0.0, np.float32(0), NEG_BIAS).astype(np.float32)

    # causal triangle for the diagonal 128x128 blocks: keep iff q_local >= k_local
    tri = (np.arange(DH)[None, :] >= np.arange(DH)[:, None]).astype(np.float32)

    ones_c = np.ones((DH, 1), np.float32)
    ones_r = np.ones((1, DH), np.float32)

    in_maps = []
    for c in range(NCORES):
        b, hg = divmod(c, 2)
        e0 = hg * EH
        in_maps.append(
            {
                "qT": qT[b],
                "kT": kT[b],
                "wqT": np.ascontiguousarray(WqT[:, e0 : e0 + EH]),
                "wkT": np.ascontiguousarray(WkT[:, e0 : e0 + EH]),
                "wvT": np.ascontiguousarray(WvT[:, e0 : e0 + EH]),
                "bq_d": np.ascontiguousarray(bq[e0 : e0 + EH].reshape(HPC, DH).T),
                "bk_d": np.ascontiguousarray(bk[e0 : e0 + EH].reshape(HPC, DH).T),
                "bv_d": np.ascontiguousarray(bv[e0 : e0 + EH].reshape(1, EH)),
                "kbias_d": np.ascontiguousarray(kbias_all[b].reshape(NKT, DH).T),
                "tri_d": tri,
                "ones_c_d": ones_c,
                "ones_r_d": ones_r,
                "resid_d": np.ascontiguousarray(qT[b][e0 : e0 + EH, :]),
            }
        )
    return in_maps


def _assemble(results):
    """results: list of 8 dicts with outT_d [EH, S] -> full [B, S, HID]."""
    out = np.empty((B, S, HID), np.float32)
    for c in range(NCORES):
        b, hg = divmod(c, 2)
        out[b, :, hg * EH : (hg + 1) * EH] = results[c]["outT_d"].T
    return out


def kernel(**inputs):
    from concourse.bass_utils import run_bass_kernel_spmd

    nc = _build()
    in_maps = _host_prep(**inputs)
    res = run_bass_kernel_spmd(nc, in_maps, core_ids=list(range(NCORES)))
    kernel.last_results = res
    return _assemble(res.results)
